# revision 4
# baseline (speedup 1.0000x reference)
"""Trainium2 Bass kernel: basic GCN layer, row-parallel over 8 NeuronCores.

    Y = relu( D^-1/2 (A + I) D^-1/2 (H @ W.T + b) ),  D = (A + I).sum(axis=1)

Core i owns output rows [i*1024, (i+1)*1024). (A+I)[rows].T is staged in
HBM pre-tiled to the SBUF layout [p, k, r] (p = contraction lane, k = 64
k-tiles, r = 1024 local rows) in fp8 (lossless for a 0/1/2 matrix) and
stays fp8 in SBUF, halving DMA traffic and enabling DoubleRow (dual-row
fp8) matmuls at 2x bf16 PE throughput. The identity is folded into A on
the host, so row sums give D directly and the main matmul includes the
self-loop term.

Schedule: the load window runs fp8 DoubleRow PE work (row sums over A
chunks as they land, with warm fillers holding the PE p-state); a dummy
collective at kernel start absorbs the runtime's first-collective
rendezvous during the load. D^-1/2 is computed in a [8, 128] layout
(multi-partition, so sqrt + reciprocal are ~cheap) and all-gathered. The
H @ W.T + b precompute (bf16) fills the all-gather wait. The main matmul
runs fp8 DoubleRow with two accumulation passes - X quantized to fp8
plus its fp8 residual - which recovers bf16-class accuracy at 2x speed.
X-prep uses 4 grouped whole-tile DVE/ACT ops (broadcast_to on the
per-column scale) instead of per-k-tile ops.
"""

import os
import sys

import numpy as np

for _p in ("/opt/trn_rl_repo", "/root/.axon_site/_ro/trn_rl_repo"):
    if _p not in sys.path and os.path.isdir(_p):
        sys.path.insert(0, _p)

N = 8192        # nodes
NCORES = 8
RPC = N // NCORES  # rows per core (1024)
P = 128         # partitions / tile edge
F = 128         # feature dim (in == out)
KT = N // P     # k-tiles (64)
NCH = 16        # A load chunks
KPC = KT // NCH  # k-tiles per chunk (4)
WARM0 = 6       # PE warm-up matmuls at kernel start (dual fp8)
WFILL = 2       # dual-fp8 warm fillers per chunk during the load
NG = 4          # X-prep groups in the tail
GT = KT // NG   # k-tiles per group (16)


def _build_nc(ncores=NCORES, warm0=WARM0, wfill=WFILL, dummy_ag=True):
    import concourse.bass as bass  # noqa: F401
    import concourse.mybir as mybir
    from concourse import bacc, tile
    from concourse.masks import make_identity

    dt = mybir.dt
    f32, bf, f8 = dt.float32, dt.bfloat16, dt.float8e4
    DR = mybir.MatmulPerfMode.DoubleRow
    Alu = mybir.AluOpType

    nc = bacc.Bacc("TRN2", num_devices=ncores)

    at = nc.dram_tensor("at", [P, KT * RPC], f8, kind="ExternalInput")
    ht = nc.dram_tensor("ht", [F, N], bf, kind="ExternalInput")
    wt = nc.dram_tensor("wt", [F, F], bf, kind="ExternalInput")
    bias = nc.dram_tensor("bias", [1, F], bf, kind="ExternalInput")
    out = nc.dram_tensor("out", [F, RPC], f32, kind="ExternalOutput")

    with tile.TileContext(nc) as tc:
        with (
            tc.tile_pool(name="const", bufs=1) as cpool,
            tc.tile_pool(name="abuf", bufs=1) as apool,
            tc.tile_pool(name="xbuf", bufs=1) as xpool,
            tc.tile_pool(name="work", bufs=1) as wpool,
            tc.tile_pool(name="tmp", bufs=4) as tpool,
            tc.tile_pool(name="pshw", bufs=2, space="PSUM") as pshw,
            tc.tile_pool(name="psbig", bufs=1, space="PSUM") as psbig,
            tc.tile_pool(name="dram", bufs=1, space="DRAM") as dpool,
        ):
            # ---- constants / small loads (scalar queue) ----
            wt_sb = cpool.tile([F, F], bf, tag="wt", name="wt_sb")
            bias_bc = cpool.tile([P, F], bf, tag="biasbc", name="bias_bc")
            ht_sb = cpool.tile([F, N], bf, tag="ht", name="ht_sb")
            ones2 = cpool.tile([P, 2, P], f8, tag="ones2", name="ones2")
            ident = cpool.tile([P, P], f32, tag="ident", name="ident")
            nc.scalar.dma_start(wt_sb[:], wt[:])
            nc.scalar.dma_start(
                bias_bc[:].rearrange("p (o f) -> p o f", o=1),
                bias[0:1, :].partition_broadcast(P),
            )
            nc.vector.memset(ones2[:], 1.0)
            make_identity(nc, ident[:])

            # pre-warm the ACT sqrt table so the real sqrt is table-hit
            tw = wpool.tile([1, 8], f32, tag="tw", name="tw")
            nc.vector.memset(tw[:], 1.0)
            nc.scalar.sqrt(tw[:], tw[:])

            # ---- dummy collective: absorb first-CC rendezvous early ----
            if dummy_ag:
                dsrc = wpool.tile([1, 8], f32, tag="dsrc", name="dsrc")
                dumb = wpool.tile([8, 8], f32, tag="dumb", name="dumb")
                di = dpool.tile([1, 8], f32, tag="di", name="di")
                do_ = dpool.tile([8, 8], f32, tag="do", name="do",
                                 addr_space="Shared")
                nc.vector.memset(dsrc[:], 0.0)
                nc.sync.dma_start(di[:], dsrc[:])
                nc.gpsimd.collective_compute(
                    "AllGather", mybir.AluOpType.bypass,
                    replica_groups=[list(range(ncores))],
                    ins=[di[:].opt()], outs=[do_[:].opt()],
                )
                nc.sync.dma_start(dumb[:], do_[:])

            # ---- A load; rowsums + warm fillers, all dual-fp8 ----
            # Rowsum weights are a full [128, 2, 128] all-ones tile: every
            # output row equals the rowsum vector (we read row 0). Matmul
            # cost only scales with output columns, and small (<=2 col)
            # dual-fp8 Ldweights intermittently fail walrus's
            # s3_lw_dual_fp8_restrictions check, so big weights are both
            # free and the only shape observed to always compile.
            a_all = apool.tile([P, KT, RPC], f8, tag="a", name="a_all")
            ps_rs = [psbig.tile([P, 512], f32, tag=f"rs{rc}", name=f"rs{rc}")
                     for rc in range(2)]
            ps_warm = pshw.tile([P, 512], f32, tag="hw", name="ps_warm")
            HTC = N // NCH  # ht chunk cols (512)
            for g in range(NCH):
                nc.gpsimd.dma_start(
                    a_all[:, g * KPC:(g + 1) * KPC, :],
                    at[:, g * KPC * RPC:(g + 1) * KPC * RPC].rearrange(
                        "p (k r) -> p k r", k=KPC),
                )
            # ht chunks load after A on the same queue (fills the AG wait)
            for g in range(NCH):
                nc.gpsimd.dma_start(ht_sb[:, g * HTC:(g + 1) * HTC],
                                    ht[:, g * HTC:(g + 1) * HTC])

            def warm_dr(n):
                for _ in range(n):
                    nc.tensor.matmul(ps_warm[:, :], ones2[:, :, :],
                                     a_all[:, 0:2, 0:512],
                                     start=True, stop=True, perf_mode=DR)

            def absorb_wait(k):
                # A plain (non-dual) fp8 matmul touches a fresh chunk
                # first, absorbing the chunk-DMA semaphore wait away from
                # the dual-fp8 Ldweights that follow.
                nc.tensor.matmul(
                    ps_warm[0:2, 0:2],
                    ones2[:, 0:1, 0:2].rearrange("p k m -> p (k m)"),
                    a_all[:, k:k + 1, 0:2].rearrange("p k r -> p (k r)"),
                    start=True, stop=True,
                )

            absorb_wait(0)
            warm_dr(warm0)
            for g in range(NCH):
                if g > 0:
                    absorb_wait(g * KPC)
                for pp in range(KPC // 2):
                    k0 = g * KPC + 2 * pp
                    for rc in range(2):
                        nc.tensor.matmul(
                            ps_rs[rc][:, :], ones2[:, :, :],
                            a_all[:, k0:k0 + 2, rc * 512:(rc + 1) * 512],
                            start=(g == 0 and pp == 0),
                            stop=(g == NCH - 1 and pp == KPC // 2 - 1),
                            perf_mode=DR,
                        )
                if g > 0:
                    warm_dr(wfill)

            # ---- local D^-1/2 in [8, 128] layout; kick the all-gather ----
            # (I is folded into A on the host, so rowsums give D directly
            # and D >= 1 always.)
            rs_sb = wpool.tile([1, RPC], f32, tag="rs", name="rs_sb")
            for rc in range(2):
                nc.vector.tensor_copy(rs_sb[0:1, rc * 512:(rc + 1) * 512],
                                      ps_rs[rc][0:1, :])
            # partition-spread must bounce through DRAM (flat addressing);
            # an SBUF rearrange cannot move data across partitions
            rsd = dpool.tile([1, RPC], f32, tag="rsd", name="rsd")
            nc.sync.dma_start(rsd[:], rs_sb[:])
            rs8 = wpool.tile([8, P], f32, tag="rs8", name="rs8")
            nc.sync.dma_start(
                rs8[:], rsd[0:1, :].rearrange("o (g p) -> (o g) p", g=8))
            dv8 = wpool.tile([8, P], f32, tag="dv8", name="dv8")
            nc.scalar.sqrt(dv8[:], rs8[:])
            nc.vector.reciprocal(dv8[:], dv8[:])

            ci_g = dpool.tile([8, P], f32, tag="cig", name="ci_g")
            co = dpool.tile([8 * ncores, P], f32, tag="co", name="co",
                            addr_space="Shared")
            nc.sync.dma_start(ci_g[:], dv8[:])
            nc.gpsimd.collective_compute(
                "AllGather", mybir.AluOpType.bypass,
                replica_groups=[list(range(ncores))],
                ins=[ci_g[:].opt()], outs=[co[:].opt()],
            )

            # dlb[p, r] = dinv_local[r] (output-side scaling; off critical
            # path - only needed at the epilogue)
            dvd = dpool.tile([1, RPC], f32, tag="dvd", name="dvd")
            dlb = wpool.tile([P, RPC], f32, tag="dlb", name="dlb")
            nc.sync.dma_start(
                dvd[0:1, :].rearrange("o (g p) -> (o g) p", g=8), dv8[:])
            nc.sync.dma_start(
                dlb[:].rearrange("p (o r) -> p o r", o=1),
                dvd[0:1, :].partition_broadcast(P),
            )

            # ---- AG window: HW = H @ W.T + b (bf16) for all k-tiles ----
            hwb = xpool.tile([P, KT, F], bf, tag="hwb", name="hwb")
            for k in range(KT):
                ps_hw = pshw.tile([P, F], f32, tag="hw", name=f"hw{k}")
                nc.tensor.matmul(ps_hw[:, :],
                                 ht_sb[:, k * P:(k + 1) * P],
                                 wt_sb[:, :], start=True, stop=True)
                nc.vector.tensor_add(hwb[:, k, :], ps_hw[:, :],
                                     bias_bc[:, :])

            # gathered dinv -> dall [p, k] (k = global k-tile)
            cosb = wpool.tile([8 * ncores, P], f32, tag="cosb", name="cosb")
            dall = wpool.tile([P, KT], f32, tag="dall", name="dall")
            nc.sync.dma_start(cosb[:], co[:])
            ps_t1 = pshw.tile([P, KT], f32, tag="hw", name="ps_t1")
            nc.tensor.transpose(ps_t1[:, :], cosb[:, :],
                                ident[0:KT, 0:KT])
            nc.vector.tensor_copy(dall[:], ps_t1[:, :])

            # ---- tail: grouped scale+quantize X, fp8 DoubleRow main mm ----
            t_x = xpool.tile([P, KT, F], f32, tag="tx", name="t_x")
            xhi = xpool.tile([P, KT, F], f8, tag="xhi", name="xhi")
            xres = xpool.tile([P, KT, F], f8, tag="xres", name="xres")
            ps_main = [psbig.tile([F, 512], f32, tag=f"main{rc}",
                                  name=f"main{rc}") for rc in range(2)]
            for gi in range(NG):
                sl = slice(gi * GT, (gi + 1) * GT)
                dbc = dall[:, sl].unsqueeze(2).broadcast_to([P, GT, F])
                nc.vector.tensor_mul(t_x[:, sl, :], hwb[:, sl, :], dbc)
                nc.scalar.copy(xhi[:, sl, :], t_x[:, sl, :])
                nc.vector.tensor_sub(xres[:, sl, :], t_x[:, sl, :],
                                     xhi[:, sl, :])
                for pp in range(GT // 2):
                    k0 = gi * GT + 2 * pp
                    for rc in range(2):
                        rhs = a_all[:, k0:k0 + 2, rc * 512:(rc + 1) * 512]
                        nc.tensor.matmul(ps_main[rc][:, :],
                                         xhi[:, k0:k0 + 2, :],
                                         rhs,
                                         start=(gi == 0 and pp == 0),
                                         stop=False, perf_mode=DR)
                        nc.tensor.matmul(ps_main[rc][:, :],
                                         xres[:, k0:k0 + 2, :],
                                         rhs, start=False,
                                         stop=(gi == NG - 1
                                               and pp == GT // 2 - 1),
                                         perf_mode=DR)

            # -- epilogue: Y.T = relu(main) * dlb --
            y_sb = wpool.tile([F, RPC], f32, tag="y", name="y_sb")
            for rc in range(2):
                dsl = dlb[:, rc * 512:(rc + 1) * 512]
                nc.vector.scalar_tensor_tensor(
                    y_sb[:, rc * 512:(rc + 1) * 512], ps_main[rc][:, :],
                    0.0, dsl, op0=Alu.max, op1=Alu.mult)
                nc.sync.dma_start(out[:, rc * 512:(rc + 1) * 512],
                                  y_sb[:, rc * 512:(rc + 1) * 512])

    nc.compile()
    return nc


_CACHE = {}


def _get_nc():
    if "nc" not in _CACHE:
        _CACHE["nc"] = _build_nc()
    return _CACHE["nc"]


def _prep_in_maps(H, A, W, b):
    import ml_dtypes

    bf16 = ml_dtypes.bfloat16
    fp8 = ml_dtypes.float8_e4m3
    H = np.asarray(H, dtype=np.float32)
    A = np.asarray(A, dtype=np.float32)
    W = np.asarray(W, dtype=np.float32)
    b = np.asarray(b, dtype=np.float32)
    Hb = H.astype(bf16)
    ht = np.ascontiguousarray(Hb.T)
    wt = np.ascontiguousarray(W.T.astype(bf16))
    bias = np.ascontiguousarray(b.reshape(1, -1).astype(bf16))
    maps = []
    for i in range(NCORES):
        r0 = i * RPC
        # A+I slice: fold the self-loop into the staged adjacency
        Asl = A[r0:r0 + RPC, :].copy()
        Asl[np.arange(RPC), r0 + np.arange(RPC)] += 1.0
        Ab = Asl.astype(fp8)
        # at[p, k*RPC + r] = (A+I)[row0 + r, k*128 + p]
        at = np.ascontiguousarray(
            Ab.T.reshape(KT, P, RPC).transpose(1, 0, 2).reshape(
                P, KT * RPC))
        maps.append({
            "at": at,
            "ht": ht,
            "wt": wt,
            "bias": bias,
        })
    return maps


def run(H, A, W, b, trace=False):
    from concourse import bass_utils

    nc = _get_nc()
    res = bass_utils.run_bass_kernel_spmd(
        nc, _prep_in_maps(H, A, W, b), core_ids=list(range(NCORES)),
        trace=trace,
    )
    Y = np.concatenate(
        [np.asarray(res.results[i]["out"]).T for i in range(NCORES)], axis=0
    )
    return np.ascontiguousarray(Y, dtype=np.float32), res


def kernel(H, A, W, b):
    return run(H, A, W, b)[0]


# revision 5
# speedup vs baseline: 1.5049x; 1.5049x over previous
"""Trainium2 Bass kernel: basic GCN layer, row-parallel over 8 NeuronCores.

    Y = relu( D^-1/2 (A + I) D^-1/2 (H @ W.T + b) ),  D = (A + I).sum(axis=1)

Core i owns output rows [i*1024, (i+1)*1024). (A+I)[rows].T is staged in
HBM pre-tiled to the SBUF layout [p, k, r] (p = contraction lane, k = 64
k-tiles, r = 1024 local rows) in fp8 (lossless for a 0/1/2 matrix) and
stays fp8 in SBUF, halving DMA traffic and enabling DoubleRow (dual-row
fp8) matmuls at 2x bf16 PE throughput. The identity is folded into A on
the host, so row sums give D directly and the main matmul includes the
self-loop term.

Schedule: the load window runs fp8 DoubleRow PE work (row sums over A
chunks as they land, with warm fillers holding the PE p-state); a dummy
collective at kernel start absorbs the runtime's first-collective
rendezvous during the load. D^-1/2 is computed in a [8, 128] layout
(multi-partition, so sqrt + reciprocal are ~cheap) and all-gathered. The
H @ W.T + b precompute (bf16) fills the all-gather wait. The main matmul
runs fp8 DoubleRow with two accumulation passes - X quantized to fp8
plus its fp8 residual - which recovers bf16-class accuracy at 2x speed.
X-prep uses 4 grouped whole-tile DVE/ACT ops (broadcast_to on the
per-column scale) instead of per-k-tile ops.
"""

import os
import sys

import numpy as np

for _p in ("/opt/trn_rl_repo", "/root/.axon_site/_ro/trn_rl_repo"):
    if _p not in sys.path and os.path.isdir(_p):
        sys.path.insert(0, _p)

N = 8192        # nodes
NCORES = 8
RPC = N // NCORES  # rows per core (1024)
P = 128         # partitions / tile edge
F = 128         # feature dim (in == out)
KT = N // P     # k-tiles (64)
NCH = 16        # A load chunks
KPC = KT // NCH  # k-tiles per chunk (4)
WARM0 = 6       # PE warm-up matmuls at kernel start (dual fp8)
WFILL = 2       # dual-fp8 warm fillers per chunk during the load
NG = 4          # X-prep groups in the tail
GT = KT // NG   # k-tiles per group (16)


def _build_nc(ncores=NCORES, warm0=WARM0, wfill=WFILL, dummy_ag=True):
    import concourse.bass as bass  # noqa: F401
    import concourse.mybir as mybir
    from concourse import bacc, tile
    from concourse.masks import make_identity

    dt = mybir.dt
    f32, bf, f8 = dt.float32, dt.bfloat16, dt.float8e4
    DR = mybir.MatmulPerfMode.DoubleRow
    Alu = mybir.AluOpType

    nc = bacc.Bacc("TRN2", num_devices=ncores)

    at = nc.dram_tensor("at", [P, KT * RPC], f8, kind="ExternalInput")
    ht = nc.dram_tensor("ht", [F, N], bf, kind="ExternalInput")
    wt = nc.dram_tensor("wt", [F, F], bf, kind="ExternalInput")
    bias = nc.dram_tensor("bias", [1, F], bf, kind="ExternalInput")
    out = nc.dram_tensor("out", [F, RPC], f32, kind="ExternalOutput")

    with tile.TileContext(nc) as tc:
        with (
            tc.tile_pool(name="const", bufs=1) as cpool,
            tc.tile_pool(name="abuf", bufs=1) as apool,
            tc.tile_pool(name="xbuf", bufs=1) as xpool,
            tc.tile_pool(name="work", bufs=1) as wpool,
            tc.tile_pool(name="tmp", bufs=4) as tpool,
            tc.tile_pool(name="pshw", bufs=2, space="PSUM") as pshw,
            tc.tile_pool(name="psbig", bufs=1, space="PSUM") as psbig,
            tc.tile_pool(name="dram", bufs=1, space="DRAM") as dpool,
        ):
            # ---- constants / small loads (scalar queue) ----
            wt_sb = cpool.tile([F, F], bf, tag="wt", name="wt_sb")
            bias_bc = cpool.tile([P, F], bf, tag="biasbc", name="bias_bc")
            ht_sb = cpool.tile([F, N], bf, tag="ht", name="ht_sb")
            ones2 = cpool.tile([P, 2, P], f8, tag="ones2", name="ones2")
            ident = cpool.tile([P, P], f32, tag="ident", name="ident")
            nc.scalar.dma_start(wt_sb[:], wt[:])
            nc.scalar.dma_start(
                bias_bc[:].rearrange("p (o f) -> p o f", o=1),
                bias[0:1, :].partition_broadcast(P),
            )
            nc.vector.memset(ones2[:], 1.0)
            make_identity(nc, ident[:])

            # pre-warm the ACT sqrt table so the real sqrt is table-hit
            tw = wpool.tile([1, 8], f32, tag="tw", name="tw")
            nc.vector.memset(tw[:], 1.0)
            nc.scalar.sqrt(tw[:], tw[:])

            # ---- dummy collective: absorb first-CC rendezvous early ----
            if dummy_ag:
                # No drain of the dummy output: a drain DMA on the sync
                # queue would make every later sync-queue DMA (the whole
                # D^-1/2 chain) wait for the dummy AG to complete.
                dsrc = wpool.tile([1, 8], f32, tag="dsrc", name="dsrc")
                di = dpool.tile([1, 8], f32, tag="di", name="di")
                do_ = dpool.tile([8, 8], f32, tag="do", name="do",
                                 addr_space="Shared")
                nc.vector.memset(dsrc[:], 0.0)
                nc.sync.dma_start(di[:], dsrc[:])
                nc.gpsimd.collective_compute(
                    "AllGather", mybir.AluOpType.bypass,
                    replica_groups=[list(range(ncores))],
                    ins=[di[:].opt()], outs=[do_[:].opt()],
                )

            # ---- A load; rowsums + warm fillers, all dual-fp8 ----
            # Rowsum weights are a full [128, 2, 128] all-ones tile: every
            # output row equals the rowsum vector (we read row 0). Matmul
            # cost only scales with output columns, and small (<=2 col)
            # dual-fp8 Ldweights intermittently fail walrus's
            # s3_lw_dual_fp8_restrictions check, so big weights are both
            # free and the only shape observed to always compile.
            a_all = apool.tile([P, KT, RPC], f8, tag="a", name="a_all")
            ps_rs = [psbig.tile([P, 512], f32, tag=f"rs{rc}", name=f"rs{rc}")
                     for rc in range(2)]
            ps_warm = pshw.tile([P, 512], f32, tag="hw", name="ps_warm")
            HTC = N // NCH  # ht chunk cols (512)
            for g in range(NCH):
                nc.gpsimd.dma_start(
                    a_all[:, g * KPC:(g + 1) * KPC, :],
                    at[:, g * KPC * RPC:(g + 1) * KPC * RPC].rearrange(
                        "p (k r) -> p k r", k=KPC),
                )
            # ht chunks load after A on the same queue (fills the AG wait)
            for g in range(NCH):
                nc.gpsimd.dma_start(ht_sb[:, g * HTC:(g + 1) * HTC],
                                    ht[:, g * HTC:(g + 1) * HTC])

            def warm_dr(n):
                for _ in range(n):
                    nc.tensor.matmul(ps_warm[:, :], ones2[:, :, :],
                                     a_all[:, 0:2, 0:512],
                                     start=True, stop=True, perf_mode=DR)

            def absorb_wait(k):
                # A plain (non-dual) fp8 matmul touches a fresh chunk
                # first, absorbing the chunk-DMA semaphore wait away from
                # the dual-fp8 Ldweights that follow.
                nc.tensor.matmul(
                    ps_warm[0:2, 0:2],
                    ones2[:, 0:1, 0:2].rearrange("p k m -> p (k m)"),
                    a_all[:, k:k + 1, 0:2].rearrange("p k r -> p (k r)"),
                    start=True, stop=True,
                )

            absorb_wait(0)
            warm_dr(warm0)
            for g in range(NCH):
                if g > 0:
                    absorb_wait(g * KPC)
                for pp in range(KPC // 2):
                    k0 = g * KPC + 2 * pp
                    for rc in range(2):
                        nc.tensor.matmul(
                            ps_rs[rc][:, :], ones2[:, :, :],
                            a_all[:, k0:k0 + 2, rc * 512:(rc + 1) * 512],
                            start=(g == 0 and pp == 0),
                            stop=(g == NCH - 1 and pp == KPC // 2 - 1),
                            perf_mode=DR,
                        )
                if g > 0:
                    warm_dr(wfill)

            # ---- local D^-1/2 in [8, 128] layout; kick the all-gather ----
            # (I is folded into A on the host, so rowsums give D directly
            # and D >= 1 always.)
            rs_sb = wpool.tile([1, RPC], f32, tag="rs", name="rs_sb")
            for rc in range(2):
                nc.vector.tensor_copy(rs_sb[0:1, rc * 512:(rc + 1) * 512],
                                      ps_rs[rc][0:1, :])
            # partition-spread must bounce through DRAM (flat addressing);
            # an SBUF rearrange cannot move data across partitions
            rsd = dpool.tile([1, RPC], f32, tag="rsd", name="rsd")
            nc.sync.dma_start(rsd[:], rs_sb[:])
            rs8 = wpool.tile([8, P], f32, tag="rs8", name="rs8")
            nc.sync.dma_start(
                rs8[:], rsd[0:1, :].rearrange("o (g p) -> (o g) p", g=8))
            dv8 = wpool.tile([8, P], f32, tag="dv8", name="dv8")
            nc.scalar.sqrt(dv8[:], rs8[:])
            nc.vector.reciprocal(dv8[:], dv8[:])

            ci_g = dpool.tile([8, P], f32, tag="cig", name="ci_g")
            co = dpool.tile([8 * ncores, P], f32, tag="co", name="co",
                            addr_space="Shared")
            nc.sync.dma_start(ci_g[:], dv8[:])
            nc.gpsimd.collective_compute(
                "AllGather", mybir.AluOpType.bypass,
                replica_groups=[list(range(ncores))],
                ins=[ci_g[:].opt()], outs=[co[:].opt()],
            )

            # dlb[p, r] = dinv_local[r] (output-side scaling; off critical
            # path - only needed at the epilogue)
            dvd = dpool.tile([1, RPC], f32, tag="dvd", name="dvd")
            dlb = wpool.tile([P, RPC], f32, tag="dlb", name="dlb")
            nc.sync.dma_start(
                dvd[0:1, :].rearrange("o (g p) -> (o g) p", g=8), dv8[:])
            nc.sync.dma_start(
                dlb[:].rearrange("p (o r) -> p o r", o=1),
                dvd[0:1, :].partition_broadcast(P),
            )

            # ---- AG window: HW = H @ W.T + b (bf16) for all k-tiles ----
            hwb = xpool.tile([P, KT, F], bf, tag="hwb", name="hwb")
            for k in range(KT):
                ps_hw = pshw.tile([P, F], f32, tag="hw", name=f"hw{k}")
                nc.tensor.matmul(ps_hw[:, :],
                                 ht_sb[:, k * P:(k + 1) * P],
                                 wt_sb[:, :], start=True, stop=True)
                nc.vector.tensor_add(hwb[:, k, :], ps_hw[:, :],
                                     bias_bc[:, :])

            # gathered dinv -> dall [p, k] (k = global k-tile)
            cosb = wpool.tile([8 * ncores, P], f32, tag="cosb", name="cosb")
            dall = wpool.tile([P, KT], f32, tag="dall", name="dall")
            nc.sync.dma_start(cosb[:], co[:])
            ps_t1 = pshw.tile([P, KT], f32, tag="hw", name="ps_t1")
            nc.tensor.transpose(ps_t1[:, :], cosb[:, :],
                                ident[0:KT, 0:KT])
            nc.vector.tensor_copy(dall[:], ps_t1[:, :])

            # ---- tail: grouped scale+quantize X, fp8 DoubleRow main mm ----
            t_x = xpool.tile([P, KT, F], f32, tag="tx", name="t_x")
            xhi = xpool.tile([P, KT, F], f8, tag="xhi", name="xhi")
            xres = xpool.tile([P, KT, F], f8, tag="xres", name="xres")
            ps_main = [psbig.tile([F, 512], f32, tag=f"main{rc}",
                                  name=f"main{rc}") for rc in range(2)]
            for gi in range(NG):
                sl = slice(gi * GT, (gi + 1) * GT)
                dbc = dall[:, sl].unsqueeze(2).broadcast_to([P, GT, F])
                nc.vector.tensor_mul(t_x[:, sl, :], hwb[:, sl, :], dbc)
                nc.scalar.copy(xhi[:, sl, :], t_x[:, sl, :])
                nc.vector.tensor_sub(xres[:, sl, :], t_x[:, sl, :],
                                     xhi[:, sl, :])
                for pp in range(GT // 2):
                    k0 = gi * GT + 2 * pp
                    for rc in range(2):
                        rhs = a_all[:, k0:k0 + 2, rc * 512:(rc + 1) * 512]
                        nc.tensor.matmul(ps_main[rc][:, :],
                                         xhi[:, k0:k0 + 2, :],
                                         rhs,
                                         start=(gi == 0 and pp == 0),
                                         stop=False, perf_mode=DR)
                        nc.tensor.matmul(ps_main[rc][:, :],
                                         xres[:, k0:k0 + 2, :],
                                         rhs, start=False,
                                         stop=(gi == NG - 1
                                               and pp == GT // 2 - 1),
                                         perf_mode=DR)

            # -- epilogue: Y.T = relu(main) * dlb --
            y_sb = wpool.tile([F, RPC], f32, tag="y", name="y_sb")
            for rc in range(2):
                dsl = dlb[:, rc * 512:(rc + 1) * 512]
                nc.vector.scalar_tensor_tensor(
                    y_sb[:, rc * 512:(rc + 1) * 512], ps_main[rc][:, :],
                    0.0, dsl, op0=Alu.max, op1=Alu.mult)
                nc.sync.dma_start(out[:, rc * 512:(rc + 1) * 512],
                                  y_sb[:, rc * 512:(rc + 1) * 512])

    nc.compile()
    return nc


_CACHE = {}


def _get_nc():
    if "nc" not in _CACHE:
        _CACHE["nc"] = _build_nc()
    return _CACHE["nc"]


def _prep_in_maps(H, A, W, b):
    import ml_dtypes

    bf16 = ml_dtypes.bfloat16
    fp8 = ml_dtypes.float8_e4m3
    H = np.asarray(H, dtype=np.float32)
    A = np.asarray(A, dtype=np.float32)
    W = np.asarray(W, dtype=np.float32)
    b = np.asarray(b, dtype=np.float32)
    Hb = H.astype(bf16)
    ht = np.ascontiguousarray(Hb.T)
    wt = np.ascontiguousarray(W.T.astype(bf16))
    bias = np.ascontiguousarray(b.reshape(1, -1).astype(bf16))
    maps = []
    for i in range(NCORES):
        r0 = i * RPC
        # A+I slice: fold the self-loop into the staged adjacency
        Asl = A[r0:r0 + RPC, :].copy()
        Asl[np.arange(RPC), r0 + np.arange(RPC)] += 1.0
        Ab = Asl.astype(fp8)
        # at[p, k*RPC + r] = (A+I)[row0 + r, k*128 + p]
        at = np.ascontiguousarray(
            Ab.T.reshape(KT, P, RPC).transpose(1, 0, 2).reshape(
                P, KT * RPC))
        maps.append({
            "at": at,
            "ht": ht,
            "wt": wt,
            "bias": bias,
        })
    return maps


def run(H, A, W, b, trace=False):
    from concourse import bass_utils

    nc = _get_nc()
    res = bass_utils.run_bass_kernel_spmd(
        nc, _prep_in_maps(H, A, W, b), core_ids=list(range(NCORES)),
        trace=trace,
    )
    Y = np.concatenate(
        [np.asarray(res.results[i]["out"]).T for i in range(NCORES)], axis=0
    )
    return np.ascontiguousarray(Y, dtype=np.float32), res


def kernel(H, A, W, b):
    return run(H, A, W, b)[0]


# revision 10
# speedup vs baseline: 1.5067x; 1.0012x over previous
"""Trainium2 Bass kernel: basic GCN layer, row-parallel over 8 NeuronCores.

    Y = relu( D^-1/2 (A + I) D^-1/2 (H @ W.T + b) ),  D = (A + I).sum(axis=1)

Core i owns output rows [i*1024, (i+1)*1024). (A+I)[rows].T is staged in
HBM pre-tiled to the SBUF layout [p, k, r] (p = contraction lane, k = 64
k-tiles, r = 1024 local rows) in fp8 (lossless for a 0/1/2 matrix) and
stays fp8 in SBUF, halving DMA traffic and enabling DoubleRow (dual-row
fp8) matmuls at 2x bf16 PE throughput. The identity is folded into A on
the host, so row sums give D directly and the main matmul includes the
self-loop term.

Schedule: the load window runs fp8 DoubleRow PE work (row sums over A
chunks as they land, with warm fillers holding the PE p-state); a dummy
collective at kernel start absorbs the runtime's first-collective
rendezvous during the load. D^-1/2 is computed in a [8, 128] layout
(multi-partition, so sqrt + reciprocal are ~cheap) and all-gathered. The
H @ W.T + b precompute (bf16) fills the all-gather wait. The main matmul
runs fp8 DoubleRow with two accumulation passes - X quantized to fp8
plus its fp8 residual - which recovers bf16-class accuracy at 2x speed.
X-prep uses 4 grouped whole-tile DVE/ACT ops (broadcast_to on the
per-column scale) instead of per-k-tile ops.
"""

import os
import sys

import numpy as np

for _p in ("/opt/trn_rl_repo", "/root/.axon_site/_ro/trn_rl_repo"):
    if _p not in sys.path and os.path.isdir(_p):
        sys.path.insert(0, _p)

N = 8192        # nodes
NCORES = 8
RPC = N // NCORES  # rows per core (1024)
P = 128         # partitions / tile edge
F = 128         # feature dim (in == out)
KT = N // P     # k-tiles (64)
NCH = 8         # A load chunks
KPC = KT // NCH  # k-tiles per chunk (8)
WARM0 = 2       # PE warm-up matmuls at kernel start (dual fp8)
NG = 4          # X-prep groups in the tail
GT = KT // NG   # k-tiles per group (16)


def _build_nc(ncores=NCORES, warm0=WARM0, dummy_ag=True):
    import concourse.bass as bass  # noqa: F401
    import concourse.mybir as mybir
    from concourse import bacc, tile
    from concourse.masks import make_identity

    dt = mybir.dt
    f32, bf, f8 = dt.float32, dt.bfloat16, dt.float8e4
    DR = mybir.MatmulPerfMode.DoubleRow
    Alu = mybir.AluOpType

    nc = bacc.Bacc("TRN2", num_devices=ncores)

    at = nc.dram_tensor("at", [P, KT * RPC], f8, kind="ExternalInput")
    ht = nc.dram_tensor("ht", [F, N], bf, kind="ExternalInput")
    wt = nc.dram_tensor("wt", [F, F], bf, kind="ExternalInput")
    bias = nc.dram_tensor("bias", [1, F], bf, kind="ExternalInput")
    out = nc.dram_tensor("out", [F, RPC], f32, kind="ExternalOutput")

    with tile.TileContext(nc) as tc:
        with (
            tc.tile_pool(name="const", bufs=1) as cpool,
            tc.tile_pool(name="abuf", bufs=1) as apool,
            tc.tile_pool(name="xbuf", bufs=1) as xpool,
            tc.tile_pool(name="work", bufs=1) as wpool,
            tc.tile_pool(name="tmp", bufs=4) as tpool,
            tc.tile_pool(name="pshw", bufs=2, space="PSUM") as pshw,
            tc.tile_pool(name="psbig", bufs=1, space="PSUM") as psbig,
            tc.tile_pool(name="dram", bufs=1, space="DRAM") as dpool,
        ):
            # ---- constants / small loads (scalar queue) ----
            wt_sb = cpool.tile([F, F], bf, tag="wt", name="wt_sb")
            bias_bc = cpool.tile([P, F], bf, tag="biasbc", name="bias_bc")
            ht_sb = cpool.tile([F, N], bf, tag="ht", name="ht_sb")
            ones2 = cpool.tile([P, 2, P], f8, tag="ones2", name="ones2")
            ident = cpool.tile([P, P], f32, tag="ident", name="ident")
            nc.scalar.dma_start(wt_sb[:], wt[:])
            nc.scalar.dma_start(
                bias_bc[:].rearrange("p (o f) -> p o f", o=1),
                bias[0:1, :].partition_broadcast(P),
            )
            nc.vector.memset(ones2[:], 1.0)
            make_identity(nc, ident[:])

            # pre-warm the ACT sqrt table so the real sqrt is table-hit
            tw = wpool.tile([1, 8], f32, tag="tw", name="tw")
            nc.vector.memset(tw[:], 1.0)
            nc.scalar.sqrt(tw[:], tw[:])

            # ---- dummy collective: absorb first-CC rendezvous early ----
            if dummy_ag:
                # No drain of the dummy output: a drain DMA on the sync
                # queue would make every later sync-queue DMA (the whole
                # D^-1/2 chain) wait for the dummy AG to complete.
                dsrc = wpool.tile([1, 8], f32, tag="dsrc", name="dsrc")
                di = dpool.tile([1, 8], f32, tag="di", name="di")
                do_ = dpool.tile([8, 8], f32, tag="do", name="do",
                                 addr_space="Shared")
                nc.vector.memset(dsrc[:], 0.0)
                nc.sync.dma_start(di[:], dsrc[:])
                nc.gpsimd.collective_compute(
                    "AllGather", mybir.AluOpType.bypass,
                    replica_groups=[list(range(ncores))],
                    ins=[di[:].opt()], outs=[do_[:].opt()],
                )

            # ---- A load; rowsums + warm fillers, all dual-fp8 ----
            # Rowsum weights are a full [128, 2, 128] all-ones tile: every
            # output row equals the rowsum vector (we read row 0). Matmul
            # cost only scales with output columns, and small (<=2 col)
            # dual-fp8 Ldweights intermittently fail walrus's
            # s3_lw_dual_fp8_restrictions check, so big weights are both
            # free and the only shape observed to always compile.
            a_all = apool.tile([P, KT, RPC], f8, tag="a", name="a_all")
            ps_rs = [psbig.tile([P, 512], f32, tag=f"rs{rc}", name=f"rs{rc}")
                     for rc in range(2)]
            hwb = xpool.tile([P, KT, F], bf, tag="hwb", name="hwb")
            for g in range(NCH):
                nc.gpsimd.dma_start(
                    a_all[:, g * KPC:(g + 1) * KPC, :],
                    at[:, g * KPC * RPC:(g + 1) * KPC * RPC].rearrange(
                        "p (k r) -> p k r", k=KPC),
                )
            # ht loads concurrently on the scalar queue (separate DGE)
            NHT = 16
            HTC = N // NHT  # ht chunk cols (512)
            for g in range(NHT):
                nc.scalar.dma_start(ht_sb[:, g * HTC:(g + 1) * HTC],
                                    ht[:, g * HTC:(g + 1) * HTC])

            def warm_dr(n):
                # warms into ps_rs[0]; wiped by the first start=True rowsum
                for _ in range(n):
                    nc.tensor.matmul(ps_rs[0][:, :], ones2[:, :, :],
                                     a_all[:, 0:2, 0:512],
                                     start=True, stop=True, perf_mode=DR)

            ps_hw = [pshw.tile([P, KPC, F], f32, tag="hw", name=f"hw{i}")
                     for i in range(2)]

            def absorb_wait(k, g):
                # A plain (non-dual) fp8 matmul touches a fresh chunk
                # first, absorbing the chunk-DMA semaphore wait away from
                # the dual-fp8 Ldweights that follow. Writes a corner of
                # the hw psum buffer chunk g will use; wiped by its
                # start=True matmul.
                nc.tensor.matmul(
                    ps_hw[g % 2][0:2, 0, 0:2],
                    ones2[:, 0:1, 0:2].rearrange("p k m -> p (k m)"),
                    a_all[:, k:k + 1, 0:2].rearrange("p k r -> p (k r)"),
                    start=True, stop=True,
                )

            def hw_chunk(g):
                # HW = H @ W.T for k-tiles of chunk g into one psum buf,
                # then a single grouped bias add into hwb (bf16)
                pb = ps_hw[g % 2]
                for j in range(KPC):
                    k = g * KPC + j
                    nc.tensor.matmul(pb[:, j, :],
                                     ht_sb[:, k * P:(k + 1) * P],
                                     wt_sb[:, :], start=(j == 0),
                                     stop=(j == KPC - 1))
                bb = bias_bc[:].unsqueeze(1).broadcast_to([P, KPC, F])
                nc.vector.tensor_add(
                    hwb[:, g * KPC:(g + 1) * KPC, :], pb[:, :, :], bb)

            absorb_wait(0, 0)
            warm_dr(warm0)
            for g in range(NCH):
                if g > 0:
                    absorb_wait(g * KPC, g)
                for pp in range(KPC // 2):
                    k0 = g * KPC + 2 * pp
                    for rc in range(2):
                        nc.tensor.matmul(
                            ps_rs[rc][:, :], ones2[:, :, :],
                            a_all[:, k0:k0 + 2, rc * 512:(rc + 1) * 512],
                            start=(g == 0 and pp == 0),
                            stop=(g == NCH - 1 and pp == KPC // 2 - 1),
                            perf_mode=DR,
                        )
                if g > 0:
                    # hw-precompute for the previous chunk fills the DMA
                    # wait and keeps the PE warm with real work
                    hw_chunk(g - 1)
            hw_chunk(NCH - 1)

            # ---- local D^-1/2 in [8, 128] layout; kick the all-gather ----
            # (I is folded into A on the host, so rowsums give D directly
            # and D >= 1 always.)
            rs_sb = wpool.tile([1, RPC], f32, tag="rs", name="rs_sb")
            for rc in range(2):
                nc.vector.tensor_copy(rs_sb[0:1, rc * 512:(rc + 1) * 512],
                                      ps_rs[rc][0:1, :])
            # partition-spread must bounce through DRAM (flat addressing);
            # an SBUF rearrange cannot move data across partitions
            rsd = dpool.tile([1, RPC], f32, tag="rsd", name="rsd")
            nc.sync.dma_start(rsd[:], rs_sb[:])
            rs8 = wpool.tile([8, P], f32, tag="rs8", name="rs8")
            nc.sync.dma_start(
                rs8[:], rsd[0:1, :].rearrange("o (g p) -> (o g) p", g=8))
            dv8 = wpool.tile([8, P], f32, tag="dv8", name="dv8")
            nc.scalar.sqrt(dv8[:], rs8[:])
            nc.vector.reciprocal(dv8[:], dv8[:])

            ci_g = dpool.tile([8, P], f32, tag="cig", name="ci_g")
            co = dpool.tile([8 * ncores, P], f32, tag="co", name="co",
                            addr_space="Shared")
            nc.sync.dma_start(ci_g[:], dv8[:])
            nc.gpsimd.collective_compute(
                "AllGather", mybir.AluOpType.bypass,
                replica_groups=[list(range(ncores))],
                ins=[ci_g[:].opt()], outs=[co[:].opt()],
            )

            # dlb[p, r] = dinv_local[r] (output-side scaling; off critical
            # path - only needed at the epilogue)
            dvd = dpool.tile([1, RPC], f32, tag="dvd", name="dvd")
            dlb = wpool.tile([P, RPC], f32, tag="dlb", name="dlb")
            nc.sync.dma_start(
                dvd[0:1, :].rearrange("o (g p) -> (o g) p", g=8), dv8[:])
            nc.sync.dma_start(
                dlb[:].rearrange("p (o r) -> p o r", o=1),
                dvd[0:1, :].partition_broadcast(P),
            )

            # gathered dinv -> dall [p, k] (k = global k-tile)
            cosb = wpool.tile([8 * ncores, P], f32, tag="cosb", name="cosb")
            dall = wpool.tile([P, KT], f32, tag="dall", name="dall")
            nc.sync.dma_start(cosb[:], co[:])
            ps_t1 = pshw.tile([P, KT], f32, tag="hw", name="ps_t1")
            nc.tensor.transpose(ps_t1[:, :], cosb[:, :],
                                ident[0:KT, 0:KT])
            nc.vector.tensor_copy(dall[:], ps_t1[:, :])

            # ---- tail: grouped scale+quantize X, fp8 DoubleRow main mm ----
            t_x = xpool.tile([P, KT, F], f32, tag="tx", name="t_x")
            xhi = xpool.tile([P, KT, F], f8, tag="xhi", name="xhi")
            xres = xpool.tile([P, KT, F], f8, tag="xres", name="xres")
            ps_main = [psbig.tile([F, 512], f32, tag=f"main{rc}",
                                  name=f"main{rc}") for rc in range(2)]
            for gi in range(NG):
                sl = slice(gi * GT, (gi + 1) * GT)
                dbc = dall[:, sl].unsqueeze(2).broadcast_to([P, GT, F])
                nc.vector.tensor_mul(t_x[:, sl, :], hwb[:, sl, :], dbc)
                nc.scalar.copy(xhi[:, sl, :], t_x[:, sl, :])
                nc.vector.tensor_sub(xres[:, sl, :], t_x[:, sl, :],
                                     xhi[:, sl, :])
                # all hi matmuls of the group first (they only need
                # mul+copy), then the res matmuls (need the sub too)
                for src, last in ((xhi, False), (xres, True)):
                    for pp in range(GT // 2):
                        k0 = gi * GT + 2 * pp
                        for rc in range(2):
                            rhs = a_all[:, k0:k0 + 2,
                                        rc * 512:(rc + 1) * 512]
                            nc.tensor.matmul(
                                ps_main[rc][:, :], src[:, k0:k0 + 2, :],
                                rhs,
                                start=(gi == 0 and pp == 0 and src is xhi),
                                stop=(gi == NG - 1 and last
                                      and pp == GT // 2 - 1),
                                perf_mode=DR)

            # -- epilogue: Y.T = relu(main) * dlb, 256-col chunks so the
            # out DMA overlaps the remaining compute --
            y_sb = wpool.tile([F, RPC], f32, tag="y", name="y_sb")
            for rc in range(2):
                for h in range(2):
                    c0 = rc * 512 + h * 256
                    nc.vector.scalar_tensor_tensor(
                        y_sb[:, c0:c0 + 256],
                        ps_main[rc][:, h * 256:(h + 1) * 256],
                        0.0, dlb[:, c0:c0 + 256], op0=Alu.max, op1=Alu.mult)
                    nc.sync.dma_start(out[:, c0:c0 + 256],
                                      y_sb[:, c0:c0 + 256])

    nc.compile()
    return nc


_CACHE = {}


def _get_nc():
    if "nc" not in _CACHE:
        _CACHE["nc"] = _build_nc()
    return _CACHE["nc"]


def _prep_in_maps(H, A, W, b):
    import ml_dtypes

    bf16 = ml_dtypes.bfloat16
    fp8 = ml_dtypes.float8_e4m3
    H = np.asarray(H, dtype=np.float32)
    A = np.asarray(A, dtype=np.float32)
    W = np.asarray(W, dtype=np.float32)
    b = np.asarray(b, dtype=np.float32)
    Hb = H.astype(bf16)
    ht = np.ascontiguousarray(Hb.T)
    wt = np.ascontiguousarray(W.T.astype(bf16))
    bias = np.ascontiguousarray(b.reshape(1, -1).astype(bf16))
    maps = []
    for i in range(NCORES):
        r0 = i * RPC
        # A+I slice: fold the self-loop into the staged adjacency
        Asl = A[r0:r0 + RPC, :].copy()
        Asl[np.arange(RPC), r0 + np.arange(RPC)] += 1.0
        Ab = Asl.astype(fp8)
        # at[p, k*RPC + r] = (A+I)[row0 + r, k*128 + p]
        at = np.ascontiguousarray(
            Ab.T.reshape(KT, P, RPC).transpose(1, 0, 2).reshape(
                P, KT * RPC))
        maps.append({
            "at": at,
            "ht": ht,
            "wt": wt,
            "bias": bias,
        })
    return maps


def run(H, A, W, b, trace=False):
    from concourse import bass_utils

    nc = _get_nc()
    res = bass_utils.run_bass_kernel_spmd(
        nc, _prep_in_maps(H, A, W, b), core_ids=list(range(NCORES)),
        trace=trace,
    )
    Y = np.concatenate(
        [np.asarray(res.results[i]["out"]).T for i in range(NCORES)], axis=0
    )
    return np.ascontiguousarray(Y, dtype=np.float32), res


def kernel(H, A, W, b):
    return run(H, A, W, b)[0]


# revision 11
# speedup vs baseline: 1.5702x; 1.0422x over previous
"""Trainium2 Bass kernel: basic GCN layer, row-parallel over 8 NeuronCores.

    Y = relu( D^-1/2 (A + I) D^-1/2 (H @ W.T + b) ),  D = (A + I).sum(axis=1)

Core i owns output rows [i*1024, (i+1)*1024). (A+I)[rows].T is staged in
HBM pre-tiled to the SBUF layout [p, k, r] (p = contraction lane, k = 64
k-tiles, r = 1024 local rows) in fp8 (lossless for a 0/1/2 matrix) and
stays fp8 in SBUF, halving DMA traffic and enabling DoubleRow (dual-row
fp8) matmuls at 2x bf16 PE throughput. The identity is folded into A on
the host, so row sums give D directly and the main matmul includes the
self-loop term.

Schedule: the load window runs fp8 DoubleRow PE work (row sums over A
chunks as they land, with warm fillers holding the PE p-state); a dummy
collective at kernel start absorbs the runtime's first-collective
rendezvous during the load. D^-1/2 is computed in a [8, 128] layout
(multi-partition, so sqrt + reciprocal are ~cheap) and all-gathered. The
H @ W.T + b precompute (bf16) fills the all-gather wait. The main matmul
runs fp8 DoubleRow with two accumulation passes - X quantized to fp8
plus its fp8 residual - which recovers bf16-class accuracy at 2x speed.
X-prep uses 4 grouped whole-tile DVE/ACT ops (broadcast_to on the
per-column scale) instead of per-k-tile ops.
"""

import os
import sys

import numpy as np

for _p in ("/opt/trn_rl_repo", "/root/.axon_site/_ro/trn_rl_repo"):
    if _p not in sys.path and os.path.isdir(_p):
        sys.path.insert(0, _p)

N = 8192        # nodes
NCORES = 8
RPC = N // NCORES  # rows per core (1024)
P = 128         # partitions / tile edge
F = 128         # feature dim (in == out)
KT = N // P     # k-tiles (64)
NCH = 8         # A load chunks
KPC = KT // NCH  # k-tiles per chunk (8)
WARM0 = 2       # PE warm-up matmuls at kernel start (dual fp8)
NG = 4          # X-prep groups in the tail
GT = KT // NG   # k-tiles per group (16)


def _build_nc(ncores=NCORES, warm0=WARM0, dummy_ag=True):
    import concourse.bass as bass  # noqa: F401
    import concourse.mybir as mybir
    from concourse import bacc, tile
    from concourse.masks import make_identity

    dt = mybir.dt
    f32, bf, f8 = dt.float32, dt.bfloat16, dt.float8e4
    DR = mybir.MatmulPerfMode.DoubleRow
    Alu = mybir.AluOpType

    nc = bacc.Bacc("TRN2", num_devices=ncores)

    at = nc.dram_tensor("at", [P, KT * RPC], f8, kind="ExternalInput")
    ht = nc.dram_tensor("ht", [F, N], bf, kind="ExternalInput")
    wt = nc.dram_tensor("wt", [F, F], bf, kind="ExternalInput")
    bias = nc.dram_tensor("bias", [1, F], bf, kind="ExternalInput")
    out = nc.dram_tensor("out", [F, RPC], f32, kind="ExternalOutput")

    with tile.TileContext(nc) as tc:
        with (
            tc.tile_pool(name="const", bufs=1) as cpool,
            tc.tile_pool(name="abuf", bufs=1) as apool,
            tc.tile_pool(name="xbuf", bufs=1) as xpool,
            tc.tile_pool(name="work", bufs=1) as wpool,
            tc.tile_pool(name="tmp", bufs=4) as tpool,
            tc.tile_pool(name="pshw", bufs=2, space="PSUM") as pshw,
            tc.tile_pool(name="psbig", bufs=1, space="PSUM") as psbig,
            tc.tile_pool(name="dram", bufs=1, space="DRAM") as dpool,
        ):
            # ---- constants / small loads (scalar queue) ----
            wt_sb = cpool.tile([F, F], bf, tag="wt", name="wt_sb")
            bias_bc = cpool.tile([P, F], bf, tag="biasbc", name="bias_bc")
            ht_sb = cpool.tile([F, N], bf, tag="ht", name="ht_sb")
            ones2 = cpool.tile([P, 2, P], f8, tag="ones2", name="ones2")
            ident = cpool.tile([P, P], f32, tag="ident", name="ident")
            nc.scalar.dma_start(wt_sb[:], wt[:])
            nc.scalar.dma_start(
                bias_bc[:].rearrange("p (o f) -> p o f", o=1),
                bias[0:1, :].partition_broadcast(P),
            )
            nc.vector.memset(ones2[:], 1.0)
            make_identity(nc, ident[:])

            # pre-warm the ACT sqrt table so the real sqrt is table-hit
            tw = wpool.tile([1, 8], f32, tag="tw", name="tw")
            nc.vector.memset(tw[:], 1.0)
            nc.scalar.sqrt(tw[:], tw[:])

            # ---- dummy collective: absorb first-CC rendezvous early ----
            if dummy_ag:
                # No drain of the dummy output: a drain DMA on the sync
                # queue would make every later sync-queue DMA (the whole
                # D^-1/2 chain) wait for the dummy AG to complete.
                dsrc = wpool.tile([1, 8], f32, tag="dsrc", name="dsrc")
                di = dpool.tile([1, 8], f32, tag="di", name="di")
                do_ = dpool.tile([8, 8], f32, tag="do", name="do",
                                 addr_space="Shared")
                nc.vector.memset(dsrc[:], 0.0)
                nc.sync.dma_start(di[:], dsrc[:])
                nc.gpsimd.collective_compute(
                    "AllGather", mybir.AluOpType.bypass,
                    replica_groups=[list(range(ncores))],
                    ins=[di[:].opt()], outs=[do_[:].opt()],
                )

            # ---- A load; rowsums + warm fillers, all dual-fp8 ----
            # Rowsum weights are a full [128, 2, 128] all-ones tile: every
            # output row equals the rowsum vector (we read row 0). Matmul
            # cost only scales with output columns, and small (<=2 col)
            # dual-fp8 Ldweights intermittently fail walrus's
            # s3_lw_dual_fp8_restrictions check, so big weights are both
            # free and the only shape observed to always compile.
            a_all = apool.tile([P, KT, RPC], f8, tag="a", name="a_all")
            ps_rs = [psbig.tile([P, 512], f32, tag=f"rs{rc}", name=f"rs{rc}")
                     for rc in range(2)]
            hwb = xpool.tile([P, KT, F], bf, tag="hwb", name="hwb")
            for g in range(NCH):
                nc.gpsimd.dma_start(
                    a_all[:, g * KPC:(g + 1) * KPC, :],
                    at[:, g * KPC * RPC:(g + 1) * KPC * RPC].rearrange(
                        "p (k r) -> p k r", k=KPC),
                )
            # ht loads concurrently on the scalar queue (separate DGE)
            NHT = 16
            HTC = N // NHT  # ht chunk cols (512)
            for g in range(NHT):
                nc.scalar.dma_start(ht_sb[:, g * HTC:(g + 1) * HTC],
                                    ht[:, g * HTC:(g + 1) * HTC])

            def warm_dr(n):
                # warms into ps_rs[0]; wiped by the first start=True rowsum
                for _ in range(n):
                    nc.tensor.matmul(ps_rs[0][:, :], ones2[:, :, :],
                                     a_all[:, 0:2, 0:512],
                                     start=True, stop=True, perf_mode=DR)

            ps_hw = [pshw.tile([P, KPC, F], f32, tag="hw", name=f"hw{i}")
                     for i in range(2)]

            def absorb_wait(k, g):
                # A plain (non-dual) fp8 matmul touches a fresh chunk
                # first, absorbing the chunk-DMA semaphore wait away from
                # the dual-fp8 Ldweights that follow. Writes a corner of
                # the hw psum buffer chunk g will use; wiped by its
                # start=True matmul.
                nc.tensor.matmul(
                    ps_hw[g % 2][0:2, 0, 0:2],
                    ones2[:, 0:1, 0:2].rearrange("p k m -> p (k m)"),
                    a_all[:, k:k + 1, 0:2].rearrange("p k r -> p (k r)"),
                    start=True, stop=True,
                )

            def hw_chunk(g):
                # HW = H @ W.T for k-tiles of chunk g into one psum buf,
                # then a single grouped bias add into hwb (bf16)
                pb = ps_hw[g % 2]
                for j in range(KPC):
                    k = g * KPC + j
                    # each j writes its own psum slice - no accumulation
                    nc.tensor.matmul(pb[:, j, :],
                                     ht_sb[:, k * P:(k + 1) * P],
                                     wt_sb[:, :], start=True, stop=True)
                bb = bias_bc[:].unsqueeze(1).broadcast_to([P, KPC, F])
                nc.vector.tensor_add(
                    hwb[:, g * KPC:(g + 1) * KPC, :], pb[:, :, :], bb)

            absorb_wait(0, 0)
            warm_dr(warm0)
            for g in range(NCH):
                if g > 0:
                    absorb_wait(g * KPC, g)
                for pp in range(KPC // 2):
                    k0 = g * KPC + 2 * pp
                    for rc in range(2):
                        nc.tensor.matmul(
                            ps_rs[rc][:, :], ones2[:, :, :],
                            a_all[:, k0:k0 + 2, rc * 512:(rc + 1) * 512],
                            start=(g == 0 and pp == 0),
                            stop=(g == NCH - 1 and pp == KPC // 2 - 1),
                            perf_mode=DR,
                        )
                if g > 0:
                    # hw-precompute for the previous chunk fills the DMA
                    # wait and keeps the PE warm with real work
                    hw_chunk(g - 1)
            hw_chunk(NCH - 1)

            # ---- local D^-1/2 in [8, 128] layout; kick the all-gather ----
            # (I is folded into A on the host, so rowsums give D directly
            # and D >= 1 always.)
            rs_sb = wpool.tile([1, RPC], f32, tag="rs", name="rs_sb")
            for rc in range(2):
                nc.vector.tensor_copy(rs_sb[0:1, rc * 512:(rc + 1) * 512],
                                      ps_rs[rc][0:1, :])
            # partition-spread must bounce through DRAM (flat addressing);
            # an SBUF rearrange cannot move data across partitions
            rsd = dpool.tile([1, RPC], f32, tag="rsd", name="rsd")
            nc.sync.dma_start(rsd[:], rs_sb[:])
            rs8 = wpool.tile([8, P], f32, tag="rs8", name="rs8")
            nc.sync.dma_start(
                rs8[:], rsd[0:1, :].rearrange("o (g p) -> (o g) p", g=8))
            dv8 = wpool.tile([8, P], f32, tag="dv8", name="dv8")
            nc.scalar.sqrt(dv8[:], rs8[:])
            nc.vector.reciprocal(dv8[:], dv8[:])

            ci_g = dpool.tile([8, P], f32, tag="cig", name="ci_g")
            co = dpool.tile([8 * ncores, P], f32, tag="co", name="co",
                            addr_space="Shared")
            nc.sync.dma_start(ci_g[:], dv8[:])
            nc.gpsimd.collective_compute(
                "AllGather", mybir.AluOpType.bypass,
                replica_groups=[list(range(ncores))],
                ins=[ci_g[:].opt()], outs=[co[:].opt()],
            )

            # dlb[p, r] = dinv_local[r] (output-side scaling; off critical
            # path - only needed at the epilogue)
            dvd = dpool.tile([1, RPC], f32, tag="dvd", name="dvd")
            dlb = wpool.tile([P, RPC], f32, tag="dlb", name="dlb")
            nc.sync.dma_start(
                dvd[0:1, :].rearrange("o (g p) -> (o g) p", g=8), dv8[:])
            nc.sync.dma_start(
                dlb[:].rearrange("p (o r) -> p o r", o=1),
                dvd[0:1, :].partition_broadcast(P),
            )

            # gathered dinv -> dall [p, k] (k = global k-tile)
            cosb = wpool.tile([8 * ncores, P], f32, tag="cosb", name="cosb")
            dall = wpool.tile([P, KT], f32, tag="dall", name="dall")
            nc.sync.dma_start(cosb[:], co[:])
            ps_t1 = pshw.tile([P, KT], f32, tag="hw", name="ps_t1")
            nc.tensor.transpose(ps_t1[:, :], cosb[:, :],
                                ident[0:KT, 0:KT])
            nc.vector.tensor_copy(dall[:], ps_t1[:, :])

            # ---- tail: grouped scale+quantize X, fp8 DoubleRow main mm ----
            t_x = xpool.tile([P, KT, F], f32, tag="tx", name="t_x")
            xhi = xpool.tile([P, KT, F], f8, tag="xhi", name="xhi")
            xres = xpool.tile([P, KT, F], f8, tag="xres", name="xres")
            ps_main = [psbig.tile([F, 512], f32, tag=f"main{rc}",
                                  name=f"main{rc}") for rc in range(2)]
            for gi in range(NG):
                sl = slice(gi * GT, (gi + 1) * GT)
                dbc = dall[:, sl].unsqueeze(2).broadcast_to([P, GT, F])
                nc.vector.tensor_mul(t_x[:, sl, :], hwb[:, sl, :], dbc)
                nc.scalar.copy(xhi[:, sl, :], t_x[:, sl, :])
                nc.vector.tensor_sub(xres[:, sl, :], t_x[:, sl, :],
                                     xhi[:, sl, :])
                # all hi matmuls of the group first (they only need
                # mul+copy), then the res matmuls (need the sub too)
                for src, last in ((xhi, False), (xres, True)):
                    for pp in range(GT // 2):
                        k0 = gi * GT + 2 * pp
                        for rc in range(2):
                            rhs = a_all[:, k0:k0 + 2,
                                        rc * 512:(rc + 1) * 512]
                            nc.tensor.matmul(
                                ps_main[rc][:, :], src[:, k0:k0 + 2, :],
                                rhs,
                                start=(gi == 0 and pp == 0 and src is xhi),
                                stop=(gi == NG - 1 and last
                                      and pp == GT // 2 - 1),
                                perf_mode=DR)

            # -- epilogue: Y.T = relu(main) * dlb, 256-col chunks so the
            # out DMA overlaps the remaining compute --
            y_sb = wpool.tile([F, RPC], f32, tag="y", name="y_sb")
            for rc in range(2):
                for h in range(2):
                    c0 = rc * 512 + h * 256
                    nc.vector.scalar_tensor_tensor(
                        y_sb[:, c0:c0 + 256],
                        ps_main[rc][:, h * 256:(h + 1) * 256],
                        0.0, dlb[:, c0:c0 + 256], op0=Alu.max, op1=Alu.mult)
                    nc.sync.dma_start(out[:, c0:c0 + 256],
                                      y_sb[:, c0:c0 + 256])

    nc.compile()
    return nc


_CACHE = {}


def _get_nc():
    if "nc" not in _CACHE:
        _CACHE["nc"] = _build_nc()
    return _CACHE["nc"]


def _prep_in_maps(H, A, W, b):
    import ml_dtypes

    bf16 = ml_dtypes.bfloat16
    fp8 = ml_dtypes.float8_e4m3
    H = np.asarray(H, dtype=np.float32)
    A = np.asarray(A, dtype=np.float32)
    W = np.asarray(W, dtype=np.float32)
    b = np.asarray(b, dtype=np.float32)
    Hb = H.astype(bf16)
    ht = np.ascontiguousarray(Hb.T)
    wt = np.ascontiguousarray(W.T.astype(bf16))
    bias = np.ascontiguousarray(b.reshape(1, -1).astype(bf16))
    maps = []
    for i in range(NCORES):
        r0 = i * RPC
        # A+I slice: fold the self-loop into the staged adjacency
        Asl = A[r0:r0 + RPC, :].copy()
        Asl[np.arange(RPC), r0 + np.arange(RPC)] += 1.0
        Ab = Asl.astype(fp8)
        # at[p, k*RPC + r] = (A+I)[row0 + r, k*128 + p]
        at = np.ascontiguousarray(
            Ab.T.reshape(KT, P, RPC).transpose(1, 0, 2).reshape(
                P, KT * RPC))
        maps.append({
            "at": at,
            "ht": ht,
            "wt": wt,
            "bias": bias,
        })
    return maps


def run(H, A, W, b, trace=False):
    from concourse import bass_utils

    nc = _get_nc()
    res = bass_utils.run_bass_kernel_spmd(
        nc, _prep_in_maps(H, A, W, b), core_ids=list(range(NCORES)),
        trace=trace,
    )
    Y = np.concatenate(
        [np.asarray(res.results[i]["out"]).T for i in range(NCORES)], axis=0
    )
    return np.ascontiguousarray(Y, dtype=np.float32), res


def kernel(H, A, W, b):
    return run(H, A, W, b)[0]
